# revision 27
# baseline (speedup 1.0000x reference)
"""Trainium2 Bass kernel for CausalCrossAttention (B=8, T=769, C=1024, H=16).

Sharding: data-parallel over batch B=8 across the 8 NeuronCores (one batch
element per core, SPMD).

v2 design (vs the 340us baseline):
  * All matmul operands in bf16 (PE speed is the same 1 col/cycle as fp32r,
    but DMA bytes halve and DVE elementwise ops run in 2x packed mode).
    Accumulation stays fp32 in PSUM; rel-err budget measured at ~5e-3 in a
    host simulation vs the 2e-2 gate.
  * ACT (scalar engine) runs ONLY the softmax exp (its ~88us is the global
    pacing constraint); every PSUM eviction moved to DVE copies.
  * Emission is hand-woven so the in-order PE queue never blocks long:
    Q/K co-tiles, V tiles, PV and the output projection are spliced between
    attention score matmuls at sub-microsecond granularity, which also keeps
    the PE HAM activity monitor warm (no >3.4us gaps -> stays at 2.4 GHz).
  * PSUM budget (8 banks): one shared projection/PV pool (2 tiles x 2 banks)
    + two score tiles (2 x 2 banks).
  * V is stored ones-augmented per head pair as [a_dims(64), a_den, b_den,
    b_dims(64)] so both heads' PV outputs land partition-aligned for their
    yT eviction (cross-partition compute ops don't compile), and the
    softmax denominator falls out of the PV matmul. Denominator rows go
    PSUM->DRAM by DMA, come back as a partition-broadcast, one approx
    reciprocal + two in-place muls normalize yT.
  * Output projection in [c_out, t] layout (per-partition bias would be
    free; zero biases skip it entirely) -> contiguous DMA; host transposes.
"""

import os

import numpy as np

B, T, C = 8, 769, 1024
H, HD, L = 16, 64, 32
COND = 256
NCI = 8
NTT = 7
TP = 770
PW = 2 * HD + 2  # 130: per-pair augmented V width [a(64), a_den, b_den, b(64)]
VW = 8 * PW      # 1040

# Per-(kv-tile) allowed q ranges in the 0:512 block + mask offset.
R0SUB = {0: (0, 512, None), 1: (0, 512, None), 2: (0, 512, 0),
         3: (128, 512, 128), 4: (256, 512, 256), 5: (384, 512, 384)}

_CACHE = {}


def _build_program(use_bias):
    import concourse.mybir as mybir
    import concourse.tile as tile
    from concourse import bacc

    f32 = mybir.dt.float32
    bf16 = mybir.dt.bfloat16
    Exp = mybir.ActivationFunctionType.Exp

    nc = bacc.Bacc("TRN2", target_bir_lowering=False)

    xqT_d = nc.dram_tensor("xqT", [C, TP], bf16, kind="ExternalInput")
    xkvT_d = nc.dram_tensor("xkvT", [C, TP], bf16, kind="ExternalInput")
    wq_d = nc.dram_tensor("wqT", [C, C], bf16, kind="ExternalInput")
    wk_d = nc.dram_tensor("wkT", [C, C], bf16, kind="ExternalInput")
    wv_d = nc.dram_tensor("wvT", [C, C], bf16, kind="ExternalInput")
    wp_d = nc.dram_tensor("wpT", [C, C], bf16, kind="ExternalInput")
    cos_d = nc.dram_tensor("cosP", [128, TP], bf16, kind="ExternalInput")
    sin_d = nc.dram_tensor("sinP", [128, TP], bf16, kind="ExternalInput")
    m0_d = nc.dram_tensor("m0", [128, 128], bf16, kind="ExternalInput")
    if use_bias:
        bq_d = nc.dram_tensor("bq2", [128, NCI], f32, kind="ExternalInput")
        bk_d = nc.dram_tensor("bk2", [128, NCI], f32, kind="ExternalInput")
        bp_d = nc.dram_tensor("bp2", [128, NCI], f32, kind="ExternalInput")
        bv_d = nc.dram_tensor("bv1", [1, C], f32, kind="ExternalInput")
    out_d = nc.dram_tensor("out", [C, TP], f32, kind="ExternalOutput")

    with tile.TileContext(nc) as tc:
        with (
            tc.tile_pool(name="consts", bufs=1) as consts,
            tc.tile_pool(name="wq", bufs=1) as wqp,
            tc.tile_pool(name="wk", bufs=1) as wkp,
            tc.tile_pool(name="wv", bufs=1) as wvp,
            tc.tile_pool(name="xq", bufs=1) as xqp,
            tc.tile_pool(name="xkv", bufs=1) as xkp,
            tc.tile_pool(name="qk", bufs=1) as qkp,
            tc.tile_pool(name="vpool", bufs=1) as vpool,
            tc.tile_pool(name="ypool", bufs=1) as ypool,
            tc.tile_pool(name="shp", bufs=2) as shp,
            tc.tile_pool(name="ptp", bufs=3) as ptp,
            tc.tile_pool(name="rdp", bufs=2) as rdp,
            tc.tile_pool(name="oout", bufs=2) as ooutp,
            tc.tile_pool(name="psP", bufs=2, space="PSUM") as psP,
            tc.tile_pool(name="psS", bufs=1, space="PSUM") as psS,
            tc.tile_pool(name="dram", bufs=1, space="DRAM") as dram_pool,
        ):
            # ---------- constants + inputs ----------
            cos_sb = consts.tile([128, TP], bf16, tag="cos")
            sin_sb = consts.tile([128, TP], bf16, tag="sin")
            m0_sb = consts.tile([128, 128], bf16, tag="m0")
            nc.scalar.dma_start(out=cos_sb, in_=cos_d[:, :])
            nc.scalar.dma_start(out=sin_sb, in_=sin_d[:, :])
            nc.scalar.dma_start(out=m0_sb, in_=m0_d[:, :])
            if use_bias:
                bq_sb = consts.tile([128, NCI], f32, tag="bq")
                bk_sb = consts.tile([128, NCI], f32, tag="bk")
                bp_sb = consts.tile([128, NCI], f32, tag="bp")
                bv_sb = consts.tile([128, C], f32, tag="bv")
                nc.scalar.dma_start(out=bq_sb, in_=bq_d[:, :])
                nc.scalar.dma_start(out=bk_sb, in_=bk_d[:, :])
                nc.scalar.dma_start(out=bp_sb, in_=bp_d[:, :])
                nc.gpsimd.dma_start(
                    out=bv_sb, in_=bv_d[0:1, :].broadcast_to((128, C)))

            xq = xqp.tile([128, NCI, TP], bf16, tag="xq")
            xkv = xkp.tile([128, NCI, TP], bf16, tag="xkv")
            # inputs split even/odd ci across the sync and gpsimd DMA queues
            for x, xd in ((xq, xqT_d), (xkv, xkvT_d)):
                for ci in range(NCI):
                    q = nc.sync if ci % 2 == 0 else nc.gpsimd
                    q.dma_start(
                        out=x[:, ci, :], in_=xd[ci * 128:(ci + 1) * 128, :])

            def load_w(pool, wdram, pfx, queue, order=(0, 1)):
                ws = [[None, None] for _ in range(NCI)]
                for hf in order:
                    for ci in range(NCI):
                        wt = pool.tile([128, 512], bf16, tag=f"{pfx}{ci}h{hf}")
                        q = queue if not isinstance(queue, tuple) else queue[hf]
                        q.dma_start(
                            out=wt,
                            in_=wdram[ci * 128:(ci + 1) * 128,
                                      hf * 512:(hf + 1) * 512])
                        ws[ci][hf] = wt
                return ws

            # scalar queue: wq h0, wk h0 first (first-need order), h1 after;
            # wv split across the sync/gpsimd queues behind the x inputs
            wq = load_w(wqp, wq_d, "wq", nc.scalar, order=(0,))
            wk = load_w(wkp, wk_d, "wk", nc.scalar, order=(0,))
            wq2 = load_w(wqp, wq_d, "wq", nc.scalar, order=(1,))
            wk2 = load_w(wkp, wk_d, "wk", nc.scalar, order=(1,))
            for ci in range(NCI):
                wq[ci][1] = wq2[ci][1]
                wk[ci][1] = wk2[ci][1]
            wv = load_w(wvp, wv_d, "wv", (nc.sync, nc.gpsimd))
            # wp reuses wq's buffers (loaded late, after Q/K finish with them)
            wp = []

            qT = qkp.tile([128, NCI, TP], bf16, tag="qT")
            kT = qkp.tile([128, NCI, TP], bf16, tag="kT")
            vaug = vpool.tile([128, NTT, VW], bf16, tag="vaug")
            yT = ypool.tile([128, NCI, TP], bf16, tag="yT")
            # persistent qz double-buffers; complementary halves zeroed once
            qza = qkp.tile([128, 2, TP], bf16, tag="qza")
            qzb = qkp.tile([128, 2, TP], bf16, tag="qzb")
            nc.vector.memset(qza[64:128, :, :], 0.0)
            nc.vector.memset(qzb[0:64, :, :], 0.0)

            # warmup: throwaway matmuls paced by the arriving xq DMA tiles,
            # so the HAM clock gate releases (1.2 -> 2.4 GHz) and STAYS
            # released through the input-DMA phase
            wrm = consts.tile([128, 128], bf16, tag="wrm")
            nc.vector.memset(wrm, 0.0078125)
            wps = psS.tile([128, 1024], f32, tag="st0")
            for ci in range(NCI):
                for _ in range(2):
                    nc.tensor.matmul(wps[:, 0:512], wrm[:, 0:128],
                                     xq[:, ci, 0:512], start=True, stop=True)
            stg = rdp.tile([128, 2, TP], f32, tag="stg", bufs=1)
            dnd = dram_pool.tile([H, TP], f32, tag="dnd")

            # ---------- chunk emitters (generators yield per PE quantum) ----
            def gen_qk(which, j):
                """Q or K projection for co tile j + rotary tail."""
                w, x, outT = ((wq, xq, qT) if which == "q" else (wk, xkv, kT))
                ps = psP.tile([128, 1024], f32, tag="ps")
                for cig in range(4):  # 4 quanta of 2ci x 2 matmuls
                    for ci in (2 * cig, 2 * cig + 1):
                        lhs = w[ci][j // 4][:, (j % 4) * 128:(j % 4 + 1) * 128]
                        nc.tensor.matmul(ps[:, 0:512], lhs, x[:, ci, 0:512],
                                         start=(ci == 0), stop=(ci == 7))
                        nc.tensor.matmul(ps[:, 512:770], lhs, x[:, ci, 512:770],
                                         start=(ci == 0), stop=(ci == 7))
                    yield
                # eviction + rotary (DVE + small SBUF->SBUF swap DMAs)
                if use_bias:
                    b_sb = bq_sb if which == "q" else bk_sb
                    nc.vector.tensor_scalar_add(
                        outT[:, j, :], ps[:, 0:770], b_sb[:, j:j + 1])
                else:
                    nc.vector.tensor_copy(outT[:, j, :], ps[:, 0:770])
                sh = shp.tile([128, TP], bf16, tag="sh")
                nc.sync.dma_start(out=sh[32:64, :], in_=outT[32:64, j, :])
                for s in (0, 64):
                    nc.sync.dma_start(
                        out=sh[s:s + 16, :], in_=outT[s + 16:s + 32, j, :])
                    nc.sync.dma_start(
                        out=sh[s + 16:s + 32, :], in_=outT[s:s + 16, j, :])
                nc.vector.tensor_mul(sh[0:96, :], sh[0:96, :], sin_sb[0:96, :])
                nc.vector.tensor_mul(outT[:, j, :], outT[:, j, :], cos_sb)
                nc.vector.tensor_add(
                    outT[0:96, j, :], outT[0:96, j, :], sh[0:96, :])
                yield

            _PTS = {}

            def gen_s(j):
                """Attention scores + exp for head pair j (heads 2j, 2j+1)."""
                # qz fill (complement halves are persistent zeros)
                sl = j % 2
                nc.vector.tensor_copy(qza[0:64, sl, :], qT[0:64, j, :])
                nc.vector.tensor_copy(qzb[64:128, sl, :], qT[64:128, j, :])
                pts = {}
                for nk in range(6):
                    qlo, qhi, moff = R0SUB[nk]
                    for e, qz in ((0, qza), (1, qzb)):
                        st = psS.tile([128, 1024], f32, tag=f"st{e}")
                        lhs = kT[:, j, nk * 128:(nk + 1) * 128]
                        nc.tensor.matmul(st[:, qlo:qhi], lhs, qz[:, sl, qlo:qhi],
                                         start=True, stop=True)
                        nc.tensor.matmul(st[:, 512:770], lhs, qz[:, sl, 512:770],
                                         start=True, stop=True)
                        # pt col c <-> q position qlo + c (variable width)
                        pt = ptp.tile([128, TP - qlo], bf16, tag=f"pt{e}_{nk}")
                        pts[(e, nk)] = pt
                        nc.scalar.activation(out=pt[:, 0:TP - qlo],
                                             in_=st[:, qlo:770],
                                             func=Exp, scale=0.125)
                        if moff is not None:
                            nc.gpsimd.tensor_mul(
                                pt[:, 0:128], pt[:, 0:128], m0_sb)
                    yield
                # kv row 768: q cols 513:769 allowed (col 0 of pt6 = q512 = 0)
                for e, qz in ((0, qza), (1, qzb)):
                    st = psS.tile([128, 1024], f32, tag=f"st{e}")
                    nc.tensor.matmul(st[0:1, 0:258], kT[:, j, 768:769],
                                     qz[:, sl, 512:770], start=True, stop=True)
                    pt6 = ptp.tile([128, 258], bf16, tag=f"pt6_{e}")
                    pts[(e, 6)] = pt6
                    nc.vector.memset(pt6[0:1, 0:1], 0.0)
                    nc.scalar.activation(out=pt6[0:1, 1:258],
                                         in_=st[0:1, 1:258],
                                         func=Exp, scale=0.125)
                yield
                _PTS[j] = pts

            def gen_v(tt):
                tsz = 128 if tt < 6 else 1
                ps = psP.tile([128, 1024], f32, tag="ps")
                for cig in range(4):
                    for ci in (2 * cig, 2 * cig + 1):
                        lhs = xkv[:, ci, tt * 128:tt * 128 + tsz]
                        for hf in (0, 1):
                            nc.tensor.matmul(
                                ps[:tsz, hf * 512:hf * 512 + 512],
                                lhs, wv[ci][hf],
                                start=(ci == 0), stop=(ci == 7))
                    yield
                va = vaug[:tsz, tt, :].rearrange("p (pr w) -> p pr w", w=PW)
                ps4 = ps[:tsz, :].rearrange("p (pr e d) -> p pr e d", e=2, d=HD)
                if use_bias:
                    bv4 = bv_sb[:tsz, :].rearrange(
                        "p (pr e d) -> p pr e d", e=2, d=HD)
                    nc.vector.tensor_add(
                        va[:, :, 0:64], ps4[:, :, 0, :], bv4[:, :, 0, :])
                    nc.vector.tensor_add(
                        va[:, :, 66:130], ps4[:, :, 1, :], bv4[:, :, 1, :])
                else:
                    nc.vector.tensor_copy(va[:, :, 0:64], ps4[:, :, 0, :])
                    nc.vector.tensor_copy(va[:, :, 66:130], ps4[:, :, 1, :])
                nc.vector.memset(va[:, :, 64:66], 1.0)
                yield

            def gen_pv(h):
                j, e = h // 2, h % 2
                vs = j * PW + (0 if e == 0 else 2)
                pts = _PTS[j]
                o = psP.tile([128, 1024], f32, tag="ps")
                for nk in range(3):
                    qlo, qhi, _ = R0SUB[nk]
                    nc.tensor.matmul(o[:, qlo:qhi], vaug[:, nk, vs:vs + 128],
                                     pts[(e, nk)][:, 0:qhi - qlo],
                                     start=(nk == 0), stop=False)
                    nc.tensor.matmul(o[:, 512:770], vaug[:, nk, vs:vs + 128],
                                     pts[(e, nk)][:, 512 - qlo:770 - qlo],
                                     start=(nk == 0), stop=False)
                yield
                for nk in range(3, 6):
                    qlo, qhi, _ = R0SUB[nk]
                    nc.tensor.matmul(o[:, qlo:qhi], vaug[:, nk, vs:vs + 128],
                                     pts[(e, nk)][:, 0:qhi - qlo],
                                     start=False, stop=False)
                    nc.tensor.matmul(o[:, 512:770], vaug[:, nk, vs:vs + 128],
                                     pts[(e, nk)][:, 512 - qlo:770 - qlo],
                                     start=False, stop=False)
                nc.tensor.matmul(o[:, 512:770], vaug[0:1, 6, vs:vs + 128],
                                 pts[(e, 6)][0:1, 0:258],
                                 start=False, stop=True)
                # evict unnormalized y (partition-aligned by construction);
                # den row staged through SBUF (DMA can't source PSUM)
                sl2 = j % 2
                if e == 0:
                    nc.vector.tensor_copy(yT[0:64, j, :], o[0:64, 0:770])
                    nc.vector.tensor_copy(stg[64:65, sl2, :], o[64:65, 0:770])
                    nc.sync.dma_start(out=dnd[h:h + 1, :],
                                      in_=stg[64:65, sl2, :])
                else:
                    nc.vector.tensor_copy(yT[64:128, j, :], o[64:128, 0:770])
                    # b_den sits at partition 63; engine APs need 32-aligned
                    # bases, so copy the aligned 32-row block and DMA row 63
                    nc.vector.tensor_copy(stg[32:64, sl2, :], o[32:64, 0:770])
                    nc.sync.dma_start(out=dnd[h:h + 1, :],
                                      in_=stg[63:64, sl2, :])
                yield

            def div_fetch(j):
                rdbc = rdp.tile([128, TP], f32, tag="rdbc")
                nc.gpsimd.dma_start(
                    out=rdbc[0:64, :],
                    in_=dnd[2 * j:2 * j + 1, :].broadcast_to((64, TP)))
                nc.gpsimd.dma_start(
                    out=rdbc[64:128, :],
                    in_=dnd[2 * j + 1:2 * j + 2, :].broadcast_to((64, TP)))
                return rdbc

            def div_apply(j, rdbc):
                # reciprocal is a DVE-only custom op; the muls are all-SBUF,
                # so they run on gpsimd to keep DVE free for PSUM evictions
                nc.vector.reciprocal_approx_fast(out=rdbc, in_=rdbc)
                nc.gpsimd.tensor_mul(yT[0:64, j, :], yT[0:64, j, :],
                                     rdbc[0:64, :])
                nc.gpsimd.tensor_mul(yT[64:128, j, :], yT[64:128, j, :],
                                     rdbc[64:128, :])

            # output projection split in two passes so the ci 0..5 partial
            # (needs only pairs 0..5 divided) fills the exp/PV tail
            oparts = {}

            def gen_opart(co):
                ps = psP.tile([128, 1024], f32, tag="ps")
                for cig in range(3):
                    for ci in (2 * cig, 2 * cig + 1):
                        lhs = wp[ci][co // 4][:, (co % 4) * 128:(co % 4 + 1) * 128]
                        nc.tensor.matmul(ps[:, 0:512], lhs, yT[:, ci, 0:512],
                                         start=(ci == 0), stop=(ci == 5))
                        nc.tensor.matmul(ps[:, 512:770], lhs, yT[:, ci, 512:770],
                                         start=(ci == 0), stop=(ci == 5))
                    yield
                # partial staged in the K-weight pool's buffers (same shape,
                # wk's readers finished long ago) -> zero extra SBUF
                opA = wkp.tile([128, 512], bf16, tag=f"wk{co}h0")
                opB = wkp.tile([128, 512], bf16, tag=f"wk{co}h1")
                oparts[co] = (opA, opB)
                nc.vector.tensor_copy(opA[:, :], ps[:, 0:512])
                nc.vector.tensor_copy(opB[:, 0:258], ps[:, 512:770])
                yield

            def gen_ofinal(co):
                ps = psP.tile([128, 1024], f32, tag="ps")
                for ci in (6, 7):
                    lhs = wp[ci][co // 4][:, (co % 4) * 128:(co % 4 + 1) * 128]
                    nc.tensor.matmul(ps[:, 0:512], lhs, yT[:, ci, 0:512],
                                     start=(ci == 6), stop=(ci == 7))
                    nc.tensor.matmul(ps[:, 512:770], lhs, yT[:, ci, 512:770],
                                     start=(ci == 6), stop=(ci == 7))
                yield
                ot = ooutp.tile([128, TP], f32, tag="ot")
                opA, opB = oparts[co]
                nc.vector.tensor_add(ot[:, 0:512], ps[:, 0:512], opA)
                nc.vector.tensor_add(ot[:, 512:770], ps[:, 512:770],
                                     opB[:, 0:258])
                if use_bias:
                    nc.vector.tensor_scalar_add(ot[:, :], ot[:, :],
                                                bp_sb[:, co:co + 1])
                nc.sync.dma_start(out=out_d[co * 128:(co + 1) * 128, :], in_=ot)
                yield

            # ---------- the weave ----------
            # Emission order IS per-engine queue order; ordering constraints:
            #   QK(j) before S(j); all V before any PV; PV pair j before
            #   S(j+3) (pt pool bufs=3); divisions trail their PV by ~one
            #   pair so the DRAM denominator roundtrip is hidden.
            def drain(g):
                for _ in g:
                    pass

            def pull(n):
                while n > 0 and fillers:
                    try:
                        next(fillers[0])
                        n -= 1
                    except StopIteration:
                        fillers.pop(0)

            drain(gen_qk("q", 0))
            drain(gen_qk("q", 1))
            drain(gen_qk("k", 0))
            drain(gen_qk("k", 1))

            fillers = [gen_v(tt) for tt in range(NTT)]
            rdbcs = {}

            for j in range(8):
                for _ in gen_s(j):
                    pull(2)
                fillers.append(gen_pv(2 * j))
                fillers.append(gen_pv(2 * j + 1))
                if j >= 3:
                    rdbcs[j - 3] = div_fetch(j - 3)
                if j >= 4:
                    div_apply(j - 4, rdbcs.pop(j - 4))
                if j + 2 <= 7:
                    drain(gen_qk("q", j + 2))
                    drain(gen_qk("k", j + 2))
                if j == 5:
                    # output-projection weights into wq's (now free) buffers
                    wp.extend(load_w(wqp, wp_d, "wq", nc.sync))

            # tail: drain remaining PVs, finish divisions; O partials (ci
            # 0..5, needing only pairs 0..5) fill the exp/PV tail gaps
            pull(4)
            rdbcs[5] = div_fetch(5)
            pull(4)
            div_apply(4, rdbcs.pop(4))
            pull(4)
            rdbcs[6] = div_fetch(6)
            pull(2)
            div_apply(5, rdbcs.pop(5))
            for co in range(8):
                for _ in gen_opart(co):
                    pull(1)
            rdbcs[7] = div_fetch(7)
            div_apply(6, rdbcs.pop(6))
            while fillers:
                pull(1)
            div_apply(7, rdbcs.pop(7))
            for co in range(8):
                drain(gen_ofinal(co))

    nc.compile()
    return nc


def _host_prep(x_q, x_kv, rotary_pos_emb, Wq, bq, Wk, bk, Wv, bv, Wp, bp,
               use_bias):
    import ml_dtypes
    bf = ml_dtypes.bfloat16
    f = np.float32
    x_q = np.asarray(x_q, f)
    x_kv = np.asarray(x_kv, f)
    freqs = np.asarray(rotary_pos_emb, f)

    # Even/odd pair-split permutation of the first 32 dims of each head, so
    # rotate_half becomes a 16-partition block swap on chip.
    perm = np.arange(C)
    for h in range(H):
        b0 = h * HD
        blk = np.empty(HD, np.int64)
        blk[0:16] = b0 + np.arange(0, 32, 2)
        blk[16:32] = b0 + np.arange(1, 32, 2)
        blk[32:64] = b0 + np.arange(32, 64)
        perm[b0:b0 + HD] = blk

    def wT(W, p=None):
        W = np.asarray(W, f)
        if p is not None:
            W = W[p, :]
        return np.ascontiguousarray(W.T).astype(bf)

    cosE = np.cos(freqs[:, 0::2]).T
    cosO = np.cos(freqs[:, 1::2]).T
    sinE = -np.sin(freqs[:, 0::2]).T
    sinO = np.sin(freqs[:, 1::2]).T
    cosP = np.ones((128, TP), f)
    sinP = np.zeros((128, TP), f)
    for s in (0, 64):
        cosP[s:s + 16, :T] = cosE
        cosP[s + 16:s + 32, :T] = cosO
        sinP[s:s + 16, :T] = sinE
        sinP[s + 16:s + 32, :T] = sinO

    p_idx = np.arange(128)[:, None]
    f_idx = np.arange(128)[None, :]
    m0 = (p_idx < f_idx).astype(f)

    shared = {
        "wqT": wT(Wq, perm),
        "wkT": wT(Wk, perm),
        "wvT": wT(Wv),
        "wpT": wT(Wp),
        "cosP": cosP.astype(bf),
        "sinP": sinP.astype(bf),
        "m0": m0.astype(bf),
    }
    if use_bias:
        bqp = np.asarray(bq, f)[perm]
        bkp = np.asarray(bk, f)[perm]
        shared["bq2"] = np.ascontiguousarray(bqp.reshape(NCI, 128).T)
        shared["bk2"] = np.ascontiguousarray(bkp.reshape(NCI, 128).T)
        shared["bp2"] = np.ascontiguousarray(
            np.asarray(bp, f).reshape(NCI, 128).T)
        shared["bv1"] = np.asarray(bv, f).reshape(1, C).copy()

    def padT(xt):
        out = np.zeros((C, TP), f)
        out[:, :T] = xt
        return out.astype(bf)

    in_maps = []
    for b in range(B):
        m = dict(shared)
        m["xqT"] = padT(x_q[b].T)
        m["xkvT"] = padT(x_kv[b].T)
        in_maps.append(m)
    return in_maps


def kernel(x_q, x_kv, rotary_pos_emb, Wq, bq, Wk, bk, Wv, bv, Wp, bp):
    from concourse.bass_utils import run_bass_kernel_spmd

    use_bias = any(np.any(np.asarray(b)) for b in (bq, bk, bv, bp))
    key = ("nc", use_bias)
    if key not in _CACHE:
        _CACHE[key] = _build_program(use_bias)
    nc = _CACHE[key]

    in_maps = _host_prep(x_q, x_kv, rotary_pos_emb,
                         Wq, bq, Wk, bk, Wv, bv, Wp, bp, use_bias)
    trace = os.environ.get("BTK_TRACE", "0") == "1"
    res = run_bass_kernel_spmd(
        nc, in_maps, core_ids=list(range(B)), trace=trace)
    _CACHE["last_result"] = res
    return np.stack(
        [np.ascontiguousarray(r["out"][:, :T].T.astype(np.float32))
         for r in res.results], axis=0)
